# revision 1
# baseline (speedup 1.0000x reference)
"""ANI AEV computer on 8 TRN2 NeuronCores (Bass/Tile), data-parallel over molecules.

Self-contained: shards (32,48) species / (32,48,3) coords over 8 cores
(4 molecules each), computes radial (64) + angular (320) AEV per atom on
device, gathers to (32,48,384).
"""
import math
import numpy as np

RCR, RCA = 5.2, 3.5
ETA_R = 16.0
SHFR = np.linspace(0.9, 5.2, 17)[:-1].astype(np.float64)
SHFA = np.linspace(0.9, 3.5, 5)[:-1].astype(np.float64)
SHFZ = (np.linspace(0.0, math.pi, 9)[:-1] + math.pi / 16.0).astype(np.float64)
CAPS = (6, 7, 6, 6)
K = sum(CAPS)                      # 25 slots
BASES = (0, 6, 13, 19)
IA, IB = np.triu_indices(K, 1)     # 300 pairs
NPAIR = len(IA)
CHUNKS = [(0, 128), (128, 256), (256, NPAIR)]
NAT = 192                          # atoms per core
EPS2 = 0.01

# fc = 0.5 - 0.5*sin(pi*(d/rc-0.5)); odd poly deg 7 for sin(pi*z), z in [-.5,.5]
_z = np.linspace(-0.5, 0.5, 20001)
SINC, *_ = np.linalg.lstsq(np.stack([_z, _z**3, _z**5, _z**7], 1),
                           np.sin(np.pi * _z), rcond=None)

# const pack layout (128, 1122): cols [jrow 48 | krow 48 | ident 96 |
#   expa 300 | expb 300 | exps 300 | buck 30]
_C_JROW, _C_KROW, _C_ID, _C_EA, _C_EB, _C_ES, _C_BK, _C_SH = 0, 48, 96, 192, 492, 792, 1092, 1122
_C_W = 1890
_BUILT = {}


def _constpack():
    cp = np.zeros((128, _C_W), np.float32)
    cp[:96, _C_JROW:_C_JROW + 48] = np.arange(48, dtype=np.float32)[None, :]
    cp[:96, _C_KROW:_C_KROW + 48] = 255.0 - np.arange(48, dtype=np.float32)[None, :]
    cp[:96, _C_ID:_C_ID + 96] = np.eye(96, dtype=np.float32)
    expa = np.zeros((K, NPAIR), np.float32)
    expb = np.zeros((K, NPAIR), np.float32)
    expa[IA, np.arange(NPAIR)] = 1.0
    expb[IB, np.arange(NPAIR)] = 1.0
    cp[:K, _C_EA:_C_EA + NPAIR] = expa
    cp[:K, _C_EB:_C_EB + NPAIR] = expb
    cp[:K, _C_ES:_C_ES + NPAIR] = expa + expb
    triu = np.zeros((4, 4), np.int64)
    s1, s2 = np.triu_indices(4)
    triu[s1, s2] = np.arange(10); triu[s2, s1] = np.arange(10)
    slot_sp = np.concatenate([np.full(CAPS[s], s) for s in range(4)])
    pair_p = triu[slot_sp[IA], slot_sp[IB]]
    for c, (c0, c1) in enumerate(CHUNKS):
        oh = (pair_p[c0:c1, None] == np.arange(10)).astype(np.float32)
        cp[:c1 - c0, _C_BK + 10 * c:_C_BK + 10 * c + 10] = oh
    shrow = np.repeat(SHFR.astype(np.float32), 48)  # (768,) f-major
    cp[:96, _C_SH:_C_SH + 768] = shrow[None, :]
    return cp


def _build():
    import sys
    sys.path.insert(0, "/opt/trn_rl_repo")
    from contextlib import ExitStack
    import concourse.tile as tile
    from concourse import bacc, mybir

    f32 = mybir.dt.float32
    bf16 = mybir.dt.bfloat16
    AF = mybir.ActivationFunctionType
    ALU = mybir.AluOpType

    nc = bacc.Bacc("TRN2", target_bir_lowering=False, debug=False, num_devices=8)
    # data pack per tile: cols [sprow 48 | spp 1 | crow 144 | ctr 3]
    P_data = nc.declare_dram_parameter("data", [2, 96, 198], f32, isOutput=False)
    P_const = nc.declare_dram_parameter("consts", [128, _C_W], f32, isOutput=False)
    P_outr = nc.declare_dram_parameter("outr", [2, 8, 768], f32, isOutput=True)
    P_outa = nc.declare_dram_parameter("outa", [8, 10, 4 * NAT], f32, isOutput=True)

    CH_N = 10
    LN2H = 0.5 * math.log(2.0)
    c1_, c3_, c5_, c7_ = [float(c) for c in SINC]

    with tile.TileContext(nc) as tc, ExitStack() as ctx:
        pool = ctx.enter_context(tc.tile_pool(name="sb", bufs=1))
        psum = ctx.enter_context(tc.tile_pool(name="ps", bufs=1, space="PSUM"))

        def sbuf(shape, tag, dt=f32):
            return pool.tile(shape, dt, name=tag, tag=tag)

        CF = sbuf([128, _C_W], "constf")
        nc.sync.dma_start(CF[:], P_const[:, :])
        jrow = CF[0:96, _C_JROW:_C_JROW + 48]
        krow = CF[0:96, _C_KROW:_C_KROW + 48]
        ident = CF[0:96, _C_ID:_C_ID + 96]
        expa = CF[0:K, _C_EA:_C_EA + NPAIR]
        expb = CF[0:K, _C_EB:_C_EB + NPAIR]
        exps = CF[0:K, _C_ES:_C_ES + NPAIR]
        buckf = CF[0:128, _C_BK:_C_BK + 30]
        buck = sbuf([128, 30], "buckb", bf16)
        nc.vector.tensor_copy(buck[:], buckf)
        half = sbuf([128, 1], "half"); nc.vector.memset(half[:], 0.5)
        ln95 = sbuf([128, 1], "ln95"); nc.vector.memset(ln95[:], math.log(0.95))
        # dummy PE op to absorb the const-DMA wait on the PE engine
        scr = psum.tile([K, 96], f32, name="scr", tag="tp")
        nc.tensor.transpose(scr[:], CF[0:96, 0:K], ident)

        SD = [sbuf([K, NAT], f"sd{c}") for c in range(CH_N)]

        for t in range(2):
            data = sbuf([96, 198], f"data{t}")
            nc.sync.dma_start(data[:], P_data[t, :, :])
            sprow = data[:, 0:48]
            sppc = data[:, 48:49]
            crow = data[:, 49:193]
            ctr = data[:, 193:196]
            mcol = data[:, 196:198]

            d2 = sbuf([96, 48], "d2")
            tmp = sbuf([96, 48], "tmp")
            for c in range(3):
                dst = d2 if c == 0 else tmp
                nc.vector.tensor_scalar(dst[:], crow[:, 48 * c:48 * c + 48],
                                        ctr[:, c:c + 1], None, ALU.subtract, ALU.bypass)
                nc.vector.tensor_tensor(dst[:], dst[:], dst[:], ALU.mult)
                if c:
                    nc.vector.tensor_tensor(d2[:], d2[:], tmp[:], ALU.add)
            dr = sbuf([96, 48], "dr")
            nc.vector.tensor_scalar(dr[:], d2[:], 1e-12, None, ALU.max, ALU.bypass)
            nc.scalar.activation(dr[:], dr[:], AF.Ln, bias=0.0, scale=1.0)
            nc.scalar.activation(dr[:], dr[:], AF.Exp, bias=0.0, scale=0.5)

            # ---- radial ----
            rmask = sbuf([96, 48], "rmask")
            nc.vector.tensor_scalar(rmask[:], d2[:], RCR * RCR, None, ALU.is_le, ALU.bypass)
            nc.vector.tensor_scalar(tmp[:], d2[:], EPS2, None, ALU.is_ge, ALU.bypass)
            nc.vector.tensor_tensor(rmask[:], rmask[:], tmp[:], ALU.mult)  # keep: angular reuses tmp pattern
            zz = sbuf([96, 48], "zz")
            z2 = sbuf([96, 48], "z2")
            h = sbuf([96, 48], "h")
            nc.vector.tensor_scalar(zz[:], dr[:], RCR, None, ALU.min, ALU.bypass)
            nc.vector.tensor_scalar(zz[:], zz[:], 1.0 / RCR, -0.5, ALU.mult, ALU.add)
            nc.vector.tensor_tensor(z2[:], zz[:], zz[:], ALU.mult)
            nc.vector.tensor_scalar(h[:], z2[:], c7_, c5_, ALU.mult, ALU.add)
            nc.vector.tensor_tensor(h[:], h[:], z2[:], ALU.mult)
            nc.vector.tensor_scalar(h[:], h[:], c3_, None, ALU.add, ALU.bypass)
            nc.vector.tensor_tensor(h[:], h[:], z2[:], ALU.mult)
            nc.vector.tensor_scalar(h[:], h[:], c1_, None, ALU.add, ALU.bypass)
            nc.vector.tensor_tensor(h[:], h[:], zz[:], ALU.mult)
            wr = sbuf([96, 48], "wr")
            nc.vector.tensor_scalar(wr[:], h[:], -0.125, 0.125, ALU.mult, ALU.add)
            nc.vector.tensor_tensor(wr[:], wr[:], rmask[:], ALU.mult)
            rplane = sbuf([96, 768], "rplane")
            rp3 = rplane[:].rearrange("p (f j) -> p f j", f=16)
            drb = dr[:].rearrange("p (o j) -> p o j", o=1).broadcast_to([96, 16, 48])
            shb = CF[0:96, _C_SH:_C_SH + 768].rearrange("p (f j) -> p f j", f=16)
            nc.vector.tensor_tensor(rp3, drb, shb, ALU.subtract)
            nc.vector.tensor_tensor(rplane[:], rplane[:], rplane[:], ALU.mult)
            nc.scalar.activation(rplane[:], rplane[:], AF.Exp, bias=0.0, scale=-ETA_R)
            wrb = wr[:].rearrange("p (o j) -> p o j", o=1).broadcast_to([96, 16, 48])
            nc.vector.tensor_tensor(rp3, rp3, wrb, ALU.mult)
            oh2 = sbuf([96, 8], "oh2")
            ohc = sbuf([96, 1], "ohc")
            for m in range(2):
                for s in range(4):
                    nc.vector.tensor_scalar(ohc[:], sppc, float(s), None,
                                            ALU.is_equal, ALU.bypass)
                    nc.vector.tensor_tensor(oh2[:, 4 * m + s:4 * m + s + 1],
                                            ohc[:], mcol[:, m:m + 1], ALU.mult)
            rps = psum.tile([8, 1024], f32, name="rps", tag="aps")
            nc.tensor.matmul(rps[:, 0:384], oh2[:], rplane[:, 0:384], start=True, stop=True)
            nc.tensor.matmul(rps[:, 512:896], oh2[:], rplane[:, 384:768], start=True, stop=True)
            rsb = sbuf([8, 768], "rsb")
            nc.vector.tensor_copy(rsb[:, 0:384], rps[:, 0:384])
            nc.vector.tensor_copy(rsb[:, 384:768], rps[:, 512:896])
            nc.sync.dma_start(P_outr[t, :, :], rsb[:])

            # ---- compaction ----
            amask = sbuf([96, 48], "amask")
            nc.vector.tensor_scalar(amask[:], d2[:], RCA * RCA, None, ALU.is_le, ALU.bypass)
            nc.vector.tensor_scalar(tmp[:], d2[:], EPS2, None, ALU.is_ge, ALU.bypass)
            nc.vector.tensor_tensor(amask[:], amask[:], tmp[:], ALU.mult)
            jl = sbuf([96, K], "jl")
            key = sbuf([96, 48], "key")
            keyb = sbuf([96, 48], "keyb")
            v8 = sbuf([96, 8], "v8")
            v8b = sbuf([96, 8], "v8b")
            for s in range(4):
                kb = key if s % 2 == 0 else keyb
                vb = v8 if s % 2 == 0 else v8b
                nc.vector.tensor_scalar(kb[:], sprow, float(s), None,
                                        ALU.is_equal, ALU.bypass)
                nc.vector.tensor_tensor(kb[:], kb[:], amask[:], ALU.mult)
                nc.vector.tensor_tensor(kb[:], kb[:], krow, ALU.mult)
                nc.vector.max(vb[:], kb[:])
                nc.vector.tensor_scalar(jl[:, BASES[s]:BASES[s] + CAPS[s]],
                                        vb[:, 0:CAPS[s]], -1.0, 255.0, ALU.mult, ALU.add)
            valid = sbuf([96, K], "valid")
            nc.vector.tensor_scalar(valid[:], jl[:], 47.5, None, ALU.is_le, ALU.bypass)
            ind = sbuf([96, K * 48], "ind")
            i3 = ind[:].rearrange("p (k j) -> p k j", j=48)
            jlb = jl[:].rearrange("p (k o) -> p k o", o=1).broadcast_to([96, K, 48])
            jrb = jrow.rearrange("p (o j) -> p o j", o=1).broadcast_to([96, K, 48])
            nc.vector.tensor_tensor(i3, jlb, jrb, ALU.is_equal)
            nf = [sbuf([96, K], f"nf{q}") for q in range(4)]
            gsrc = [crow[:, 0:48], crow[:, 48:96], crow[:, 96:144], dr[:]]
            mulbuf = sbuf([96, K * 48], "mulbuf")
            m3 = mulbuf[:].rearrange("p (k j) -> p k j", j=48)
            mulbuf2 = sbuf([96, K * 48], "mulbuf2")
            m3b = mulbuf2[:].rearrange("p (k j) -> p k j", j=48)
            for q in range(4):
                qb = gsrc[q].rearrange("p (o j) -> p o j", o=1).broadcast_to([96, K, 48])
                mdst = m3 if q % 2 == 0 else m3b
                eng = nc.gpsimd if q % 2 == 0 else nc.vector
                eng.tensor_tensor(mdst, i3, qb, ALU.mult)
                nc.vector.tensor_reduce(nf[q][:].rearrange("p (k o) -> p k o", o=1), mdst,
                                        mybir.AxisListType.X, ALU.add)
            chan = [sbuf([96, K], f"ch{c}") for c in range(CH_N)]
            for c in range(3):
                nc.vector.tensor_scalar(chan[c][:], nf[c][:], ctr[:, c:c + 1], None,
                                        ALU.subtract, ALU.bypass)
            nd = chan[3]
            nc.vector.tensor_scalar(nd[:], nf[3][:], 1e-6, None, ALU.max, ALU.bypass)
            nc.vector.tensor_tensor(chan[4][:], nd[:], nd[:], ALU.mult)
            nc.scalar.activation(chan[5][:], nd[:], AF.Ln, bias=0.0, scale=1.0)
            kz = sbuf([96, K], "kz")
            kz2 = sbuf([96, K], "kz2")
            kh = sbuf([96, K], "kh")
            nc.vector.tensor_scalar(kz[:], nd[:], RCA, None, ALU.min, ALU.bypass)
            nc.vector.tensor_scalar(kz[:], kz[:], 1.0 / RCA, -0.5, ALU.mult, ALU.add)
            nc.vector.tensor_tensor(kz2[:], kz[:], kz[:], ALU.mult)
            nc.vector.tensor_scalar(kh[:], kz2[:], c7_, c5_, ALU.mult, ALU.add)
            nc.vector.tensor_tensor(kh[:], kh[:], kz2[:], ALU.mult)
            nc.vector.tensor_scalar(kh[:], kh[:], c3_, None, ALU.add, ALU.bypass)
            nc.vector.tensor_tensor(kh[:], kh[:], kz2[:], ALU.mult)
            nc.vector.tensor_scalar(kh[:], kh[:], c1_, None, ALU.add, ALU.bypass)
            nc.vector.tensor_tensor(kh[:], kh[:], kz[:], ALU.mult)
            nc.vector.tensor_scalar(kh[:], kh[:], -0.5, 0.5, ALU.mult, ALU.add)
            nc.vector.tensor_scalar(kh[:], kh[:], 1e-30, None, ALU.max, ALU.bypass)
            lnfc = sbuf([96, K], "lnfc")
            nc.scalar.activation(lnfc[:], kh[:], AF.Ln, bias=0.0, scale=1.0)
            nc.vector.tensor_scalar(kz[:], valid[:], 1e4, -1e4, ALU.mult, ALU.add)
            nc.vector.tensor_tensor(lnfc[:], lnfc[:], kz[:], ALU.add)
            kz2b = sbuf([96, K], "kz2b")
            khb = sbuf([96, K], "khb")
            for m in range(4):
                cm = float(4.0 * SHFA[m] ** 2 + LN2H)
                kzb_ = kz2 if m % 2 == 0 else kz2b
                khb_ = kh if m % 2 == 0 else khb
                nc.vector.tensor_scalar(kzb_[:], nd[:], 2.0 * float(SHFA[m]), None,
                                        ALU.subtract, ALU.bypass)
                nc.vector.tensor_tensor(kzb_[:], kzb_[:], kzb_[:], ALU.mult)
                nc.vector.tensor_scalar(khb_[:], lnfc[:], cm, None, ALU.add, ALU.bypass)
                nc.vector.scalar_tensor_tensor(chan[6 + m][:], kzb_[:], -2.0, khb_[:],
                                               ALU.mult, ALU.add)
            for c in range(CH_N):
                tp = psum.tile([K, 96], f32, name="tp", tag="tp")
                nc.tensor.transpose(tp[:], chan[c][:], ident)
                nc.vector.tensor_copy(SD[c][:, 96 * t:96 * t + 96], tp[:])

        # ---- pair expansions (per chunk) + merged plane math ----
        names = ["xa", "ya", "za", "xb", "yb", "zb",
                 "S", "SQ2", "LD", "LW0", "LW1", "LW2", "LW3"]
        pl = {n: sbuf([128, 576], f"pl_{n}") for n in names}
        for ci, (c0, c1) in enumerate(CHUNKS):
            W = c1 - c0
            idx = 0
            for mat, chans in ((expa, (0, 1, 2)), (expb, (0, 1, 2)),
                               (exps, (3, 4, 5, 6, 7, 8, 9))):
                for c in chans:
                    p = psum.tile([W, NAT], f32, name="expp", tag="expp", bufs=3)
                    nc.tensor.matmul(p[:], mat[:, c0:c1], SD[c][:], start=True, stop=True)
                    sb = pl[names[idx]]
                    if idx % 2:
                        nc.scalar.copy(sb[0:W, 192 * ci:192 * ci + 192], p[:])
                    else:
                        nc.vector.tensor_copy(sb[0:W, 192 * ci:192 * ci + 192], p[:])
                    idx += 1
        dot = sbuf([128, 576], "dot")
        t2 = sbuf([128, 576], "t2")
        t3 = sbuf([128, 576], "t3")
        nc.vector.tensor_tensor(dot[:], pl["xa"][:], pl["xb"][:], ALU.mult)
        nc.gpsimd.tensor_tensor(t2[:], pl["ya"][:], pl["yb"][:], ALU.mult)
        nc.vector.tensor_tensor(t3[:], pl["za"][:], pl["zb"][:], ALU.mult)
        nc.vector.tensor_tensor(dot[:], dot[:], t2[:], ALU.add)
        nc.vector.tensor_tensor(dot[:], dot[:], t3[:], ALU.add)
        b4 = sbuf([128, 576], "b4")
        nc.scalar.activation(b4[:], pl["S"][:], AF.Square, bias=0.0, scale=1.0)
        nc.vector.tensor_scalar(t2[:], pl["SQ2"][:], 2.0, None, ALU.mult, ALU.bypass)
        nc.vector.scalar_tensor_tensor(b4[:], b4[:], -2.0, t2[:], ALU.mult, ALU.add)
        di = sbuf([128, 576], "di")
        nc.scalar.activation(di[:], pl["LD"][:], AF.Exp, bias=ln95[0:128, :], scale=-1.0)
        cc = sbuf([128, 576], "cc")
        nc.vector.tensor_tensor(cc[:], dot[:], di[:], ALU.mult)
        nc.vector.tensor_scalar(cc[:], cc[:], 0.95, -0.95, ALU.min, ALU.max)
        chh = sbuf([128, 576], "chh")
        shh = sbuf([128, 576], "shh")
        nc.scalar.activation(chh[:], cc[:], AF.Ln, bias=half[0:128, :], scale=0.5)
        nc.scalar.activation(chh[:], chh[:], AF.Exp, bias=0.0, scale=0.5)
        nc.scalar.activation(shh[:], cc[:], AF.Ln, bias=half[0:128, :], scale=-0.5)
        nc.scalar.activation(shh[:], shh[:], AF.Exp, bias=0.0, scale=0.5)
        t2b = sbuf([128, 576], "t2b")
        wm = []
        for m in range(4):
            tbuf = t2 if m % 2 == 0 else t2b
            nc.vector.tensor_tensor(tbuf[:], pl["LW%d" % m][:], b4[:], ALU.add)
            w_ = sbuf([128, 576], f"wm{m}", bf16)
            nc.scalar.activation(w_[:], tbuf[:], AF.Exp, bias=0.0, scale=1.0)
            wm.append(w_)
        terms = [sbuf([128, 576], f"tm{mz}", bf16) for mz in range(32)]
        for z in range(8):
            tanz = float(np.tan(SHFZ[z] / 2))
            cph = float(np.cos(SHFZ[z] / 2))
            tbuf = t2 if z % 2 == 0 else t2b
            nc.vector.scalar_tensor_tensor(tbuf[:], shh[:], tanz, chh[:],
                                           ALU.mult, ALU.add)
            nc.scalar.activation(tbuf[:], tbuf[:], AF.Ln, bias=0.0, scale=cph)
            f1 = sbuf([128, 576], f"f1{z % 2}", bf16)
            nc.scalar.activation(f1[:], tbuf[:], AF.Exp, bias=0.0, scale=64.0)
            for m in range(4):
                eng = nc.vector if m < 2 else nc.gpsimd
                eng.tensor_tensor(terms[m * 8 + z][:], wm[m][:],
                                  f1[:], ALU.mult)

        # ---- bucket contraction (mz groups of 4, bank-aligned slots) ----
        for g in range(8):
            aps = psum.tile([10, 4 * 512], f32, name="aps", tag="aps")
            a3 = aps[:].rearrange("p (mz s) -> p mz s", s=512)
            for ci, (c0, c1) in enumerate(CHUNKS):
                W = c1 - c0
                for k in range(4):
                    mz = g * 4 + k
                    nc.tensor.matmul(a3[:, k, 0:NAT],
                                     buck[0:W, 10 * ci:10 * ci + 10],
                                     terms[mz][0:W, 192 * ci:192 * ci + 192],
                                     start=(ci == 0), stop=(ci == 2))
            asb = sbuf([10, 4 * NAT], f"asb{g % 2}")
            ab3 = asb[:].rearrange("p (mz at) -> p mz at", at=NAT)
            if g % 2 == 0:
                nc.vector.tensor_copy(ab3, a3[:, :, 0:NAT])
            else:
                nc.scalar.copy(ab3, a3[:, :, 0:NAT])
            nc.sync.dma_start(P_outa[g, :, :], asb[:])

    nc.compile()
    return nc


def kernel(species, coordinates):
    import sys
    sys.path.insert(0, "/opt/trn_rl_repo")
    from concourse.bass_utils import run_bass_kernel_spmd

    species = np.asarray(species)
    coords = np.asarray(coordinates, dtype=np.float32)
    N = species.shape[0]
    if "nc" not in _BUILT:
        _BUILT["nc"] = _build()
        _BUILT["cp"] = _constpack()
    nc = _BUILT["nc"]
    cp = _BUILT["cp"]

    in_maps = []
    for c in range(8):
        sp = species[4 * c:4 * c + 4].astype(np.float32)
        co = coords[4 * c:4 * c + 4]
        data = np.zeros((2, 96, 198), np.float32)
        for t in range(2):
            for m in range(2):
                n = 2 * t + m
                rows = slice(48 * m, 48 * m + 48)
                data[t, rows, 0:48] = sp[n][None, :]
                data[t, rows, 48] = sp[n]
                data[t, rows, 49:193] = co[n].T.reshape(-1)[None, :]
                data[t, rows, 193:196] = co[n]
                data[t, rows, 196 + m] = 1.0
        in_maps.append(dict(data=data, consts=cp))

    res = run_bass_kernel_spmd(nc, in_maps, list(range(8)))
    full = np.zeros((N, 48, 384), np.float32)
    for c in range(8):
        outr = np.asarray(res.results[c]["outr"]).reshape(2, 2, 4, 16, 48)
        outa = np.asarray(res.results[c]["outa"]).reshape(8, 10, 4, NAT)
        # radial: outr[t, m, s, f, i] -> mol 2t+m, atom i, col s*16+f
        rad = outr.transpose(0, 1, 4, 2, 3).reshape(4, 48, 64)
        full[4 * c:4 * c + 4, :, 0:64] = rad
        # angular: outa[g, p, k, at] -> atom at, col 64 + p*32 + g*4 + k
        ang = outa.transpose(3, 1, 0, 2).reshape(NAT, 320)
        full[4 * c:4 * c + 4, :, 64:384] = ang.reshape(4, 48, 320)
    return full



# revision 23
# speedup vs baseline: 1.2411x; 1.2411x over previous
"""ANI AEV computer on 8 TRN2 NeuronCores (Bass/Tile), data-parallel over molecules.

v2: caps (5,5,5,5) closest-first neighbor slots (K=20, 190 slot pairs in 2
chunks of 95), unit-vector half-angle angular math, single Ln/Exp act table,
multi-engine balance. Shards (32,48) species / (32,48,3) coords over 8 cores
(4 molecules each), returns (32,48,384).
"""
import math
import numpy as np

RCR, RCA = 5.2, 3.5
ETA_R = 16.0
SHFA = np.linspace(0.9, 3.5, 5)[:-1].astype(np.float64)
SHFZ = (np.linspace(0.0, math.pi, 9)[:-1] + math.pi / 16.0).astype(np.float64)
LN2H = 0.5 * math.log(2.0)
CAPS = (5, 5, 5, 5)
K = 20
IA, IB = np.triu_indices(K, 1)
NPAIR = len(IA)                     # 190
CW = 95                             # chunk width (pairs per chunk)
NAT = 192                           # atoms per core
EPS2 = 0.01
CSEL = 13.0                         # selection key offset (> RCA^2)

# fc = 0.5 - 0.5*sin(pi*(d/rc-0.5)); odd poly deg 7 for sin(pi*z), z in [-.5,.5]
_z = np.linspace(-0.5, 0.5, 20001)
SINC, *_ = np.linalg.lstsq(np.stack([_z, _z**3, _z**5, _z**7], 1),
                           np.sin(np.pi * _z), rcond=None)

# const pack layout (128, cols)
_C_JROW = 0                          # [0:96, 48] iota j
_C_EXPD = 48                         # [0:20, 190] expa - expb
_C_EXPS = 238                        # [0:20, 190] expa + expb
_C_BK = 428                          # [0:95, 20]  bucket one-hot (2 chunks x 10)
_C_BW = 448                          # [0:128, 4]  wm exp bias (2*cm per m)
_C_BZ = 452                          # [0:128, 8]  f1 exp bias (64*ln cph_z)
_C_ID = 460                          # [0:96, 96]  identity
_C_SH = 556                          # [0:96, 768] radial shifts f-major
_C_SC = 1324                         # [0:128, 20] scalar const columns
_C_W = 1344

# scalar const column values (broadcast rows for Pool-engine affine math)
_SCVALS = None  # filled in _constpack

# data pack layout (per tile row block): [96, cols]
_D_CROW = 0                          # 144: x48|y48|z48 of own molecule
_D_CTR = 144                         # 3: own coords
_D_SPM = 147                         # 192: species one-hot row masks (4 x 48)
_D_OH8 = 339                         # 8: radial scatter one-hot (mol x species)
_D_W = 347

_BUILT = {}


def _constpack():
    cp = np.zeros((128, _C_W), np.float32)
    cp[:96, _C_JROW:_C_JROW + 48] = np.arange(48, dtype=np.float32)[None, :]
    expa = np.zeros((K, NPAIR), np.float32)
    expb = np.zeros((K, NPAIR), np.float32)
    expa[IA, np.arange(NPAIR)] = 1.0
    expb[IB, np.arange(NPAIR)] = 1.0
    for b in (0, 32, 64):   # replicate at matmul base partitions
        cp[b:b + K, _C_EXPD:_C_EXPD + NPAIR] = expa - expb
        cp[b:b + K, _C_EXPS:_C_EXPS + NPAIR] = expa + expb
    triu = np.zeros((4, 4), np.int64)
    s1, s2 = np.triu_indices(4)
    triu[s1, s2] = np.arange(10); triu[s2, s1] = np.arange(10)
    slot_sp = np.concatenate([np.full(CAPS[s], s) for s in range(4)])
    pair_p = triu[slot_sp[IA], slot_sp[IB]]
    for c in range(2):
        oh = (pair_p[CW * c:CW * c + CW, None] == np.arange(10)).astype(np.float32)
        cp[:CW, _C_BK + 10 * c:_C_BK + 10 * c + 10] = oh
    for m in range(4):
        cp[:, _C_BW + m] = 2.0 * (4.0 * SHFA[m] ** 2 + LN2H)
    for z in range(8):
        cp[:, _C_BZ + z] = 64.0 * math.log(math.cos(SHFZ[z] / 2))
    cp[:96, _C_ID:_C_ID + 96] = np.eye(96, dtype=np.float32)
    shrow = np.repeat(np.linspace(0.9, 5.2, 17)[:-1].astype(np.float32), 48)
    cp[:96, _C_SH:_C_SH + 768] = shrow[None, :]
    c1_, c3_, c5_, c7_ = [float(c) for c in SINC]
    scvals = [c7_, c5_, c3_, c1_, 1.0 / RCR, -0.5, 0.125, -0.125,
              1.0 / RCA, 0.5, 1e-30, RCR, RCA,
              -2.0 * float(SHFA[0]), -2.0 * float(SHFA[1]),
              -2.0 * float(SHFA[2]), -2.0 * float(SHFA[3]), -2.0, 2.0]
    for i, v in enumerate(scvals):
        cp[:, _C_SC + i] = v
    return cp


def _build():
    import sys
    sys.path.insert(0, "/opt/trn_rl_repo")
    from contextlib import ExitStack
    import concourse.tile as tile
    from concourse import bacc, mybir

    f32 = mybir.dt.float32
    bf16 = mybir.dt.bfloat16
    u32 = mybir.dt.uint32
    AF = mybir.ActivationFunctionType
    ALU = mybir.AluOpType

    nc = bacc.Bacc("TRN2", target_bir_lowering=False, debug=False, num_devices=8)
    P_data = nc.declare_dram_parameter("data", [2, 96, _D_W], f32, isOutput=False)
    P_const = nc.declare_dram_parameter("consts", [128, _C_W], f32, isOutput=False)
    P_outr = nc.declare_dram_parameter("outr", [2, 8, 768], f32, isOutput=True)
    P_outa = nc.declare_dram_parameter("outa", [16, 10, 384], f32, isOutput=True)

    c1_, c3_, c5_, c7_ = [float(c) for c in SINC]

    with tile.TileContext(nc) as tc, ExitStack() as ctx:
        pool = ctx.enter_context(tc.tile_pool(name="sb", bufs=1))
        psum = ctx.enter_context(tc.tile_pool(name="ps", bufs=1, space="PSUM"))

        def sbuf(shape, tag, dt=f32):
            return pool.tile(shape, dt, name=tag, tag=tag)

        _bank_n = [0]

        def pbank(p0, p1, cols):
            _bank_n[0] += 1
            tl = psum.tile([128, 512], f32, name=f"bank{_bank_n[0]}",
                           tag="bank", bufs=7)
            return tl[p0:p1, 0:cols]

        CF = sbuf([128, _C_W], "constf")
        nc.sync.dma_start(CF[:], P_const[:, :])
        jrow = CF[0:96, _C_JROW:_C_JROW + 48]

        def expd_at(b, c0):
            return CF[b:b + K, _C_EXPD + c0:_C_EXPD + c0 + CW]

        def exps_at(b, c0):
            return CF[b:b + K, _C_EXPS + c0:_C_EXPS + c0 + CW]

        ident = CF[0:96, _C_ID:_C_ID + 96]
        shrow = CF[0:96, _C_SH:_C_SH + 768]
        buckb = sbuf([CW, 20], "buckb", bf16)
        nc.vector.tensor_copy(buckb[:], CF[0:CW, _C_BK:_C_BK + 20])

        # SD: transposed slot channels at 32-partition bases;
        # cols = [tile0 96 | tile1 96]
        SDu = sbuf([84, NAT], "sdu")     # ux @0:20, uy @32:52, uz @64:84
        SDs1 = sbuf([84, NAT], "sds1")   # S @0, SQ2' @32, LW0 @64
        SDs2 = sbuf([84, NAT], "sds2")   # LW1 @0, LW2 @32, LW3 @64

        def ccol(i, n, w):
            # broadcast scalar-const column i over [n, w]
            return CF[0:n, _C_SC + i:_C_SC + i + 1].rearrange(
                "p (o c) -> p o c", o=1).broadcast_to([n, 1, w])

        def pool_affine(dst3, src3, imul, iadd, n, w):
            # dst = src * sc[imul] + sc[iadd] via two Pool tensor_tensor ops
            nc.gpsimd.tensor_tensor(dst3, src3, ccol(imul, n, w), ALU.mult)
            nc.gpsimd.tensor_tensor(dst3, dst3, ccol(iadd, n, w), ALU.add)

        def poly_sin(dst, z2buf, zbuf, tmp, n, w):
            # Pool-engine sin(pi*z) poly: (((c7*z2+c5)*z2+c3)*z2+c1)*z
            t3 = tmp[:].rearrange("p (o c) -> p o c", o=1)
            z23 = z2buf[:].rearrange("p (o c) -> p o c", o=1)
            pool_affine(t3, z23, 0, 1, n, w)
            nc.gpsimd.tensor_tensor(t3, t3, z23, ALU.mult)
            nc.gpsimd.tensor_tensor(t3, t3, ccol(2, n, w), ALU.add)
            nc.gpsimd.tensor_tensor(t3, t3, z23, ALU.mult)
            nc.gpsimd.tensor_tensor(t3, t3, ccol(3, n, w), ALU.add)
            nc.gpsimd.tensor_tensor(dst[:].rearrange("p (o c) -> p o c", o=1),
                                    t3, zbuf[:].rearrange("p (o c) -> p o c", o=1),
                                    ALU.mult)

        for t in range(2):
            data = sbuf([96, _D_W], f"data{t}")
            nc.sync.dma_start(data[:], P_data[t, :, :])
            crow = data[:, _D_CROW:_D_CROW + 144]
            ctr = data[:, _D_CTR:_D_CTR + 3]
            spm = data[:, _D_SPM:_D_SPM + 192]
            oh8 = data[:, _D_OH8:_D_OH8 + 8]

            # ---- distances ----
            sqa = sbuf([96, 48], f"sqa{t}")
            sqb = sbuf([96, 48], f"sqb{t}")
            d2 = sbuf([96, 48], f"d2{t}")
            nc.scalar.activation(sqa[:], crow[:, 0:48], AF.Square,
                                 bias=ctr[:, 0:1], scale=-1.0)
            nc.scalar.activation(sqb[:], crow[:, 48:96], AF.Square,
                                 bias=ctr[:, 1:2], scale=-1.0)
            nc.vector.tensor_tensor(d2[:], sqa[:], sqb[:], ALU.add)
            nc.scalar.activation(sqa[:], crow[:, 96:144], AF.Square,
                                 bias=ctr[:, 2:3], scale=-1.0)
            nc.vector.tensor_tensor(d2[:], d2[:], sqa[:], ALU.add)
            m2 = sqb
            nc.vector.tensor_scalar(m2[:], d2[:], 1e-12, None, ALU.max, ALU.bypass)
            ln2 = sbuf([96, 48], f"ln2{t}")
            dr = sbuf([96, 48], f"dr{t}")
            nc.scalar.activation(ln2[:], m2[:], AF.Ln, bias=0.0, scale=1.0)
            nc.scalar.activation(dr[:], ln2[:], AF.Exp, bias=0.0, scale=0.5)
            selfm = sbuf([96, 48], f"selfm{t}")
            nc.vector.tensor_scalar(selfm[:], d2[:], EPS2, None, ALU.is_ge, ALU.bypass)

            # ---- radial ----
            zz = sbuf([96, 48], f"zz{t}")
            z2 = sbuf([96, 48], f"z2{t}")
            h = sbuf([96, 48], f"h{t}")
            ptmp = sbuf([96, 48], f"ptmp{t}")
            zz3 = zz[:].rearrange("p (o c) -> p o c", o=1)
            nc.vector.tensor_scalar(zz[:], dr[:], RCR, None, ALU.min, ALU.bypass)
            pool_affine(zz3, zz3, 4, 5, 96, 48)
            nc.gpsimd.tensor_tensor(z2[:], zz[:], zz[:], ALU.mult)
            poly_sin(h, z2, zz, ptmp, 96, 48)
            wr = sbuf([96, 48], f"wr{t}")
            wr3 = wr[:].rearrange("p (o c) -> p o c", o=1)
            pool_affine(wr3, h[:].rearrange("p (o c) -> p o c", o=1), 7, 6, 96, 48)
            nc.gpsimd.tensor_tensor(wr[:], wr[:], selfm[:], ALU.mult)
            wrb = sbuf([96, 48], f"wrb{t}", bf16)
            nc.gpsimd.tensor_copy(wrb[:], wr[:])
            oh8b = sbuf([96, 8], f"oh8b{t}", bf16)
            nc.gpsimd.tensor_copy(oh8b[:], oh8)

            rp = sbuf([96, 768], f"rp{t}")
            rp3 = rp[:].rearrange("p (f j) -> p f j", f=16)
            drb = dr[:].rearrange("p (o j) -> p o j", o=1).broadcast_to([96, 16, 48])
            shb = shrow.rearrange("p (f j) -> p f j", f=16)
            nc.vector.tensor_tensor(rp3, drb, shb, ALU.subtract)
            nc.scalar.activation(rp[:], rp[:], AF.Square, bias=0.0, scale=1.0)
            rpb = sbuf([96, 768], f"rpb{t}", bf16)
            nc.scalar.activation(rpb[:], rp[:], AF.Exp, bias=0.0, scale=-ETA_R)
            rpb3 = rpb[:].rearrange("p (f j) -> p f j", f=16)
            wrb3 = wrb[:].rearrange("p (o j) -> p o j", o=1).broadcast_to([96, 16, 48])
            nc.vector.tensor_tensor(rpb3, rpb3, wrb3, ALU.mult)
            rps0 = pbank(0, 8, 384)
            rps1 = pbank(0, 8, 384)
            nc.tensor.matmul(rps0, oh8b[:], rpb[:, 0:384], start=True, stop=True)
            nc.tensor.matmul(rps1, oh8b[:], rpb[:, 384:768], start=True, stop=True)
            rsb = sbuf([8, 768], f"rsb{t}")
            nc.vector.tensor_copy(rsb[:, 0:384], rps0)
            nc.scalar.copy(rsb[:, 384:768], rps1)
            nc.gpsimd.dma_start(P_outr[t, :, :], rsb[:])

            # ---- neighbor selection ----
            t1 = sbuf([96, 48], f"t1{t}")
            w = sbuf([96, 48], f"w{t}")
            nc.vector.tensor_scalar(t1[:], d2[:], RCA * RCA, None, ALU.is_le, ALU.bypass)
            nc.vector.tensor_scalar(w[:], d2[:], -1.0, CSEL, ALU.mult, ALU.add)
            nc.vector.tensor_tensor(w[:], w[:], t1[:], ALU.mult)
            nc.vector.tensor_tensor(w[:], w[:], selfm[:], ALU.mult)
            key = sbuf([96, 48], f"key{t}")
            keyb = sbuf([96, 48], f"keyb{t}")
            mv8 = sbuf([96, 32], f"mv8{t}")
            mi8 = sbuf([96, 32], f"mi8{t}", u32)
            for s in range(4):
                kb = key if s % 2 == 0 else keyb
                nc.vector.tensor_tensor(kb[:], spm[:, 48 * s:48 * s + 48], w[:],
                                        ALU.mult)
                nc.vector.max(mv8[:, 8 * s:8 * s + 8], kb[:])
                nc.vector.max_index(mi8[:, 8 * s:8 * s + 8],
                                    mv8[:, 8 * s:8 * s + 8], kb[:])
            jlf = sbuf([96, 32], f"jlf{t}")
            nc.vector.tensor_copy(jlf[:], mi8[:])
            jlc = sbuf([96, 20], f"jlc{t}")
            mvc = sbuf([96, 20], f"mvc{t}")
            jv4 = jlf[:].rearrange("p (s q) -> p s q", s=4)
            mv4 = mv8[:].rearrange("p (s q) -> p s q", s=4)
            nc.vector.tensor_copy(jlc[:].rearrange("p (s q) -> p s q", s=4),
                                  jv4[:, :, 0:5])
            nc.vector.tensor_copy(mvc[:].rearrange("p (s q) -> p s q", s=4),
                                  mv4[:, :, 0:5])

            # ---- slot-space channels ([96, 20]) ----
            stile1 = sbuf([96, 84], f"stile1{t}")
            stile2 = sbuf([96, 84], f"stile2{t}")
            nc.gpsimd.memset(stile1[:], 0.0)
            nc.gpsimd.memset(stile2[:], 0.0)
            ds = stile1[:, 0:20]
            sq2c = stile1[:, 32:52]
            d2s = sbuf([96, 20], f"d2s{t}")
            valid = sbuf([96, 20], f"valid{t}")
            nc.vector.tensor_scalar(d2s[:], mvc[:], -1.0, CSEL, ALU.mult, ALU.add)
            nc.vector.tensor_scalar(valid[:], mvc[:], 0.5, None, ALU.is_ge, ALU.bypass)
            lnd = sbuf([96, 20], f"lnd{t}")
            nc.scalar.activation(lnd[:], d2s[:], AF.Ln, bias=0.0, scale=1.0)
            nc.scalar.activation(ds, lnd[:], AF.Exp, bias=0.0, scale=0.5)
            ivd = sbuf([96, 20], f"ivd{t}")
            nc.scalar.activation(ivd[:], lnd[:], AF.Exp, bias=0.0, scale=-0.5)
            nc.vector.tensor_scalar(sq2c, d2s[:], 2.0, None, ALU.mult, ALU.bypass)

            # gather x,y,z of selected neighbors
            i3 = sbuf([96, 20 * 48], f"i3{t}")
            i33 = i3[:].rearrange("p (k j) -> p k j", j=48)
            jlb = jlc[:].rearrange("p (k o) -> p k o", o=1).broadcast_to([96, 20, 48])
            jrb = jrow.rearrange("p (o j) -> p o j", o=1).broadcast_to([96, 20, 48])
            nc.vector.tensor_tensor(i33, jlb, jrb, ALU.is_equal)
            utile = sbuf([96, 84], f"utile{t}")
            nc.gpsimd.memset(utile[:], 0.0)
            mbufa = sbuf([96, 20 * 48], f"mbufa{t}")
            mbufb = sbuf([96, 20 * 48], f"mbufb{t}")
            g3 = sbuf([96, 20], f"g3{t}")
            for c in range(3):
                eng = nc.vector if c == 0 else nc.gpsimd
                mb = (mbufa if c % 2 == 0 else mbufb)
                mb3 = mb[:].rearrange("p (k j) -> p k j", j=48)
                cb = crow[:, 48 * c:48 * c + 48].rearrange(
                    "p (o j) -> p o j", o=1).broadcast_to([96, 20, 48])
                eng.tensor_tensor(mb3, i33, cb, ALU.mult)
                nc.vector.tensor_reduce(g3[:].rearrange("p (k o) -> p k o", o=1),
                                        mb3, mybir.AxisListType.X, ALU.add)
                uc = utile[:, 32 * c:32 * c + 20]
                nc.vector.tensor_scalar(uc, g3[:], ctr[:, c:c + 1], None,
                                        ALU.subtract, ALU.bypass)
                nc.vector.tensor_tensor(uc, uc, ivd[:], ALU.mult)

            # fc_a poly + LW channels
            az = sbuf([96, 20], f"az{t}")
            az2 = sbuf([96, 20], f"az2{t}")
            ah = sbuf([96, 20], f"ah{t}")
            aptmp = sbuf([96, 20], f"aptmp{t}")
            az3 = az[:].rearrange("p (o c) -> p o c", o=1)
            nc.vector.tensor_scalar(az[:], ds, RCA, None, ALU.min, ALU.bypass)
            pool_affine(az3, az3, 8, 5, 96, 20)
            nc.gpsimd.tensor_tensor(az2[:], az[:], az[:], ALU.mult)
            poly_sin(ah, az2, az, aptmp, 96, 20)
            kh = sbuf([96, 20], f"kh{t}")
            kh3 = kh[:].rearrange("p (o c) -> p o c", o=1)
            nc.gpsimd.tensor_tensor(kh3, ah[:].rearrange("p (o c) -> p o c", o=1),
                                    ccol(5, 96, 20), ALU.mult)
            nc.gpsimd.tensor_tensor(kh3, kh3, ccol(9, 96, 20), ALU.add)
            nc.vector.tensor_scalar(kh[:], kh[:], 1e-30, None, ALU.max, ALU.bypass)
            lnfc = sbuf([96, 20], f"lnfc{t}")
            nc.scalar.activation(lnfc[:], kh[:], AF.Ln, bias=0.0, scale=1.0)
            vkill = sbuf([96, 20], f"vkill{t}")
            nc.vector.tensor_scalar(vkill[:], valid[:], 1e4, -1e4, ALU.mult, ALU.add)
            nc.vector.tensor_tensor(lnfc[:], lnfc[:], vkill[:], ALU.add)
            lwt = sbuf([96, 20], f"lwt{t}")
            lwtb = sbuf([96, 20], f"lwtb{t}")
            lwdst = [stile1[:, 64:84], stile2[:, 0:20],
                     stile2[:, 32:52], stile2[:, 64:84]]
            for m in range(4):
                tb = lwt if m % 2 == 0 else lwtb
                if m % 2 == 0:
                    nc.vector.tensor_scalar(tb[:], ds, -2.0 * float(SHFA[m]),
                                            None, ALU.add, ALU.bypass)
                    nc.vector.tensor_tensor(tb[:], tb[:], tb[:], ALU.mult)
                    nc.vector.scalar_tensor_tensor(lwdst[m], tb[:], -2.0,
                                                   lnfc[:], ALU.mult, ALU.add)
                else:
                    tb3 = tb[:].rearrange("p (o c) -> p o c", o=1)
                    nc.gpsimd.tensor_tensor(
                        tb3, ds.rearrange("p (o c) -> p o c", o=1),
                        ccol(13 + m, 96, 20), ALU.add)
                    nc.gpsimd.tensor_tensor(tb[:], tb[:], tb[:], ALU.mult)
                    nc.gpsimd.tensor_tensor(tb3, tb3, ccol(17, 96, 20), ALU.mult)
                    nc.gpsimd.tensor_tensor(lwdst[m].rearrange(
                        "p (o c) -> p o c", o=1), tb3,
                        lnfc[:].rearrange("p (o c) -> p o c", o=1), ALU.add)

            # transpose to SD
            for src_, dst in ((utile, SDu), (stile1, SDs1), (stile2, SDs2)):
                tp = pbank(0, 84, 96)
                nc.tensor.transpose(tp, src_[:], ident)
                nc.vector.tensor_copy(dst[:, 96 * t:96 * t + 96], tp)

        # ---- pair space ----
        shh2 = sbuf([CW, 384], "shh2")
        b4 = sbuf([CW, 384], "b4")
        lwb = [sbuf([CW, 384], f"lwb{m}") for m in range(4)]
        for ci in range(2):
            c0 = CW * ci
            cs = slice(192 * ci, 192 * ci + 192)
            vd = [pbank(0, CW, 192) for c in range(3)]
            for c in range(3):
                nc.tensor.matmul(vd[c], expd_at(32 * c, c0),
                                 SDu[32 * c:32 * c + 20, :], start=True, stop=True)
            pS = pbank(0, CW, 192)
            pQ = pbank(0, CW, 192)
            nc.tensor.matmul(pS, exps_at(0, c0), SDs1[0:20, :],
                             start=True, stop=True)
            nc.tensor.matmul(pQ, exps_at(32, c0), SDs1[32:52, :],
                             start=True, stop=True)
            pL = [pbank(0, CW, 192) for m in range(4)]
            srcL = [SDs1[64:84, :], SDs2[0:20, :], SDs2[32:52, :], SDs2[64:84, :]]
            basL = [64, 0, 32, 64]
            for m in range(4):
                nc.tensor.matmul(pL[m], exps_at(basL[m], c0), srcL[m],
                                 start=True, stop=True)
            # shh2 = sum_c vd_c^2 (chunk column block)
            tq = sbuf([CW, 192], f"tq{ci}")
            tq2 = sbuf([CW, 192], f"tq2{ci}")
            nc.scalar.activation(shh2[:, cs], vd[0], AF.Square, bias=0.0, scale=1.0)
            nc.scalar.activation(tq[:], vd[1], AF.Square, bias=0.0, scale=1.0)
            nc.scalar.activation(tq2[:], vd[2], AF.Square, bias=0.0, scale=1.0)
            nc.vector.tensor_tensor(shh2[:, cs], shh2[:, cs], tq[:], ALU.add)
            nc.vector.tensor_tensor(shh2[:, cs], shh2[:, cs], tq2[:], ALU.add)
            # b4 = SQ2' - 2*S^2
            s2 = sbuf([CW, 192], f"s2{ci}")
            nc.scalar.activation(s2[:], pS, AF.Square, bias=0.0, scale=1.0)
            nc.vector.scalar_tensor_tensor(b4[:, cs], s2[:], -2.0, pQ,
                                           ALU.mult, ALU.add)
            for m in range(4):
                nc.vector.tensor_tensor(lwb[m][:, cs], pL[m], b4[:, cs], ALU.add)

        nc.vector.tensor_scalar(shh2[:], shh2[:], 0.0, 4.0, ALU.max, ALU.min)
        b975 = sbuf([128, 1], "b975")
        b025 = sbuf([128, 1], "b025")
        nc.vector.memset(b975[:], 0.975)
        nc.vector.memset(b025[:], 0.025)
        lnc = sbuf([CW, 384], "lnc")
        lns = sbuf([CW, 384], "lns")
        chp = sbuf([CW, 384], "chp")
        shp = sbuf([CW, 384], "shp")
        nc.scalar.activation(lnc[:], shh2[:], AF.Ln, bias=b975[0:CW, :],
                             scale=-0.2375)
        nc.scalar.activation(chp[:], lnc[:], AF.Exp, bias=0.0, scale=0.5)
        nc.scalar.activation(lns[:], shh2[:], AF.Ln, bias=b025[0:CW, :],
                             scale=0.2375)
        nc.scalar.activation(shp[:], lns[:], AF.Exp, bias=0.0, scale=0.5)
        wm = []
        for m in range(4):
            w_ = sbuf([CW, 384], f"wm{m}", bf16)
            nc.scalar.activation(w_[:], lwb[m][:], AF.Exp,
                                 bias=CF[0:CW, _C_BW + m:_C_BW + m + 1], scale=1.0)
            wm.append(w_)
        # terms: one big tile [95, 32*384] bf16, mz = m*8+z at cols mz*384
        terms = sbuf([CW, 32 * 384], "terms", bf16)
        tv = sbuf([CW, 384], "tv")
        tvb = sbuf([CW, 384], "tvb")
        f1 = sbuf([CW, 384], "f1a", bf16)
        f1b = sbuf([CW, 384], "f1b", bf16)
        for z in range(8):
            tanz = float(np.tan(SHFZ[z] / 2))
            tb = tv if z % 2 == 0 else tvb
            fb = f1 if z % 2 == 0 else f1b
            nc.vector.scalar_tensor_tensor(tb[:], shp[:], tanz, chp[:],
                                           ALU.mult, ALU.add)
            nc.scalar.activation(tb[:], tb[:], AF.Ln, bias=0.0, scale=1.0)
            nc.scalar.activation(fb[:], tb[:], AF.Exp,
                                 bias=CF[0:CW, _C_BZ + z:_C_BZ + z + 1], scale=64.0)
            for m in range(4):
                mz = m * 8 + z
                eng2 = nc.gpsimd if (mz % 4 == 3) else nc.vector
                eng2.tensor_tensor(terms[:, 384 * mz:384 * mz + 384],
                                   wm[m][:], fb[:], ALU.mult)

        # bucket contraction: per g, rhs = 2 mz terms strided view
        t4 = terms[:].rearrange("p (mz ci at) -> p mz ci at", mz=32, ci=2)
        for g in range(16):
            pA = pbank(0, 10, 384)
            for ci in range(2):
                nc.tensor.matmul(pA, buckb[:, 10 * ci:10 * ci + 10],
                                 t4[:, 2 * g:2 * g + 2, ci, :],
                                 start=(ci == 0), stop=(ci == 1))
            asb = sbuf([10, 384], f"asb{g % 4}")
            if g % 2:
                nc.scalar.copy(asb[:], pA)
            else:
                nc.vector.tensor_copy(asb[:], pA)
            nc.gpsimd.dma_start(P_outa[g, :, :], asb[:])

    nc.compile()
    return nc


def _pack_inputs(species, coords):
    sp = species.astype(np.int64)
    co = coords.astype(np.float32)
    in_maps = []
    cp = _BUILT["cp"]
    for c in range(8):
        data = np.zeros((2, 96, _D_W), np.float32)
        for t in range(2):
            for m in range(2):
                n = 4 * c + 2 * t + m
                rows = slice(48 * m, 48 * m + 48)
                data[t, rows, _D_CROW:_D_CROW + 144] = co[n].T.reshape(-1)[None, :]
                data[t, rows, _D_CTR:_D_CTR + 3] = co[n]
                for s in range(4):
                    data[t, rows, _D_SPM + 48 * s:_D_SPM + 48 * s + 48] = \
                        (sp[n] == s).astype(np.float32)[None, :]
                for s in range(4):
                    col = _D_OH8 + 4 * m + s
                    data[t, rows, col] = (sp[n] == s).astype(np.float32)
        in_maps.append(dict(data=data, consts=cp))
    return in_maps


def kernel(species, coordinates):
    import sys
    sys.path.insert(0, "/opt/trn_rl_repo")
    from concourse.bass_utils import run_bass_kernel_spmd

    species = np.asarray(species)
    coords = np.asarray(coordinates, dtype=np.float32)
    N = species.shape[0]
    if "nc" not in _BUILT:
        _BUILT["cp"] = _constpack()
        _BUILT["nc"] = _build()
    nc = _BUILT["nc"]

    in_maps = _pack_inputs(species, coords)
    res = run_bass_kernel_spmd(nc, in_maps, list(range(8)))
    full = np.zeros((N, 48, 384), np.float32)
    for c in range(8):
        outr = np.asarray(res.results[c]["outr"]).reshape(2, 2, 4, 16, 48)
        outa = np.asarray(res.results[c]["outa"]).reshape(16, 10, 2, 4, 48)
        # radial: outr[t, m, s, f, j] -> mol 2t+m, atom j, col s*16+f
        rad = outr.transpose(0, 1, 4, 2, 3).reshape(4, 48, 64)
        full[4 * c:4 * c + 4, :, 0:64] = rad
        # angular: outa[g, p, k, mol', j]: at = 192-atom index = (t*2+m')*48+j
        # mz = 2g+k; feature col = 64 + p*32 + mz
        ang = outa.transpose(3, 4, 1, 0, 2).reshape(4, 48, 10, 32)
        full[4 * c:4 * c + 4, :, 64:384] = ang.reshape(4, 48, 320)
    return full


# revision 24
# speedup vs baseline: 1.6048x; 1.2931x over previous
"""ANI AEV computer on 8 TRN2 NeuronCores (Bass/Tile), data-parallel over molecules.

v2: caps (5,5,5,5) closest-first neighbor slots (K=20, 190 slot pairs in 2
chunks of 95), unit-vector half-angle angular math, single Ln/Exp act table,
multi-engine balance. Shards (32,48) species / (32,48,3) coords over 8 cores
(4 molecules each), returns (32,48,384).
"""
import math
import numpy as np

RCR, RCA = 5.2, 3.5
ETA_R = 16.0
SHFA = np.linspace(0.9, 3.5, 5)[:-1].astype(np.float64)
SHFZ = (np.linspace(0.0, math.pi, 9)[:-1] + math.pi / 16.0).astype(np.float64)
LN2H = 0.5 * math.log(2.0)
CAPS = (5, 5, 5, 5)
K = 20
IA, IB = np.triu_indices(K, 1)
NPAIR = len(IA)                     # 190
CW = 95                             # chunk width (pairs per chunk)
NAT = 192                           # atoms per core
EPS2 = 0.01
CSEL = 13.0                         # selection key offset (> RCA^2)

# fc = 0.5 - 0.5*sin(pi*(d/rc-0.5)); odd poly deg 7 for sin(pi*z), z in [-.5,.5]
_z = np.linspace(-0.5, 0.5, 20001)
SINC, *_ = np.linalg.lstsq(np.stack([_z, _z**3, _z**5, _z**7], 1),
                           np.sin(np.pi * _z), rcond=None)

# const pack layout (128, cols)
_C_JROW = 0                          # [0:96, 48] iota j
_C_EXPD = 48                         # [0:20, 190] expa - expb
_C_EXPS = 238                        # [0:20, 190] expa + expb
_C_BK = 428                          # [0:95, 20]  bucket one-hot (2 chunks x 10)
_C_BW = 448                          # [0:128, 4]  wm exp bias (2*cm per m)
_C_BZ = 452                          # [0:128, 8]  f1 exp bias (64*ln cph_z)
_C_ID = 460                          # [0:96, 96]  identity
_C_SH = 556                          # [0:96, 768] radial shifts f-major
_C_SC = 1324                         # [0:128, 20] scalar const columns
_C_W = 1344

# scalar const column values (broadcast rows for Pool-engine affine math)
_SCVALS = None  # filled in _constpack

# data pack layout (per tile row block): [96, cols]
_D_CROW = 0                          # 144: x48|y48|z48 of own molecule
_D_CTR = 144                         # 3: own coords
_D_SPM = 147                         # 192: species one-hot row masks (4 x 48)
_D_OH8 = 339                         # 8: radial scatter one-hot (mol x species)
_D_W = 347

_BUILT = {}


def _constpack():
    cp = np.zeros((128, _C_W), np.float32)
    cp[:96, _C_JROW:_C_JROW + 48] = np.arange(48, dtype=np.float32)[None, :]
    expa = np.zeros((K, NPAIR), np.float32)
    expb = np.zeros((K, NPAIR), np.float32)
    expa[IA, np.arange(NPAIR)] = 1.0
    expb[IB, np.arange(NPAIR)] = 1.0
    for b in (0, 32, 64):   # replicate at matmul base partitions
        cp[b:b + K, _C_EXPD:_C_EXPD + NPAIR] = expa - expb
        cp[b:b + K, _C_EXPS:_C_EXPS + NPAIR] = expa + expb
    triu = np.zeros((4, 4), np.int64)
    s1, s2 = np.triu_indices(4)
    triu[s1, s2] = np.arange(10); triu[s2, s1] = np.arange(10)
    slot_sp = np.concatenate([np.full(CAPS[s], s) for s in range(4)])
    pair_p = triu[slot_sp[IA], slot_sp[IB]]
    for c in range(2):
        oh = (pair_p[CW * c:CW * c + CW, None] == np.arange(10)).astype(np.float32)
        cp[:CW, _C_BK + 10 * c:_C_BK + 10 * c + 10] = oh
    for m in range(4):
        cp[:, _C_BW + m] = 2.0 * (4.0 * SHFA[m] ** 2 + LN2H)
    for z in range(8):
        cp[:, _C_BZ + z] = 64.0 * math.log(math.cos(SHFZ[z] / 2))
    cp[:96, _C_ID:_C_ID + 96] = np.eye(96, dtype=np.float32)
    shrow = np.repeat(np.linspace(0.9, 5.2, 17)[:-1].astype(np.float32), 48)
    cp[:96, _C_SH:_C_SH + 768] = shrow[None, :]
    c1_, c3_, c5_, c7_ = [float(c) for c in SINC]
    scvals = [c7_, c5_, c3_, c1_, 1.0 / RCR, -0.5, 0.125, -0.125,
              1.0 / RCA, 0.5, 1e-30, RCR, RCA,
              -2.0 * float(SHFA[0]), -2.0 * float(SHFA[1]),
              -2.0 * float(SHFA[2]), -2.0 * float(SHFA[3]), -2.0, 2.0]
    for i, v in enumerate(scvals):
        cp[:, _C_SC + i] = v
    return cp


def _build():
    import sys
    sys.path.insert(0, "/opt/trn_rl_repo")
    from contextlib import ExitStack
    import concourse.tile as tile
    from concourse import bacc, mybir

    f32 = mybir.dt.float32
    bf16 = mybir.dt.bfloat16
    u32 = mybir.dt.uint32
    AF = mybir.ActivationFunctionType
    ALU = mybir.AluOpType

    nc = bacc.Bacc("TRN2", target_bir_lowering=False, debug=False, num_devices=8)
    P_data = nc.declare_dram_parameter("data", [2, 96, _D_W], f32, isOutput=False)
    P_const = nc.declare_dram_parameter("consts", [128, _C_W], f32, isOutput=False)
    P_outr = nc.declare_dram_parameter("outr", [2, 8, 768], f32, isOutput=True)
    P_outa = nc.declare_dram_parameter("outa", [16, 10, 384], f32, isOutput=True)

    c1_, c3_, c5_, c7_ = [float(c) for c in SINC]

    with tile.TileContext(nc) as tc, ExitStack() as ctx:
        pool = ctx.enter_context(tc.tile_pool(name="sb", bufs=1))
        psum = ctx.enter_context(tc.tile_pool(name="ps", bufs=1, space="PSUM"))

        def sbuf(shape, tag, dt=f32):
            return pool.tile(shape, dt, name=tag, tag=tag)

        _bank_n = [0]

        def pbank(p0, p1, cols):
            _bank_n[0] += 1
            tl = psum.tile([128, 512], f32, name=f"bank{_bank_n[0]}",
                           tag="bank", bufs=7)
            return tl[p0:p1, 0:cols]

        CF = sbuf([128, _C_W], "constf")
        nc.sync.dma_start(CF[:], P_const[:, :])
        # pin the combined Ln+Exp act table so the compiler's table-load pass
        # never needs to swap sets (Square/Copy are in every set)
        from concourse.hw_specs import get_activation_tables
        _tables = list(get_activation_tables(nc.m.arch).keys())
        _set_id = _tables.index("natural_log_exp_and_others")
        nc.scalar.add_instruction(mybir.InstLoadActFuncSet(
            name=nc.get_next_instruction_name(), ins=[], outs=[],
            act_func_set_id=_set_id))
        jrow = CF[0:96, _C_JROW:_C_JROW + 48]

        def expd_at(b, c0):
            return CF[b:b + K, _C_EXPD + c0:_C_EXPD + c0 + CW]

        def exps_at(b, c0):
            return CF[b:b + K, _C_EXPS + c0:_C_EXPS + c0 + CW]

        ident = CF[0:96, _C_ID:_C_ID + 96]
        shrow = CF[0:96, _C_SH:_C_SH + 768]
        buckb = sbuf([CW, 20], "buckb", bf16)
        nc.vector.tensor_copy(buckb[:], CF[0:CW, _C_BK:_C_BK + 20])

        # SD: transposed slot channels at 32-partition bases;
        # cols = [tile0 96 | tile1 96]
        SDu = sbuf([84, NAT], "sdu")     # ux @0:20, uy @32:52, uz @64:84
        SDs1 = sbuf([84, NAT], "sds1")   # S @0, SQ2' @32, LW0 @64
        SDs2 = sbuf([84, NAT], "sds2")   # LW1 @0, LW2 @32, LW3 @64

        def ccol(i, n, w):
            # broadcast scalar-const column i over [n, w]
            return CF[0:n, _C_SC + i:_C_SC + i + 1].rearrange(
                "p (o c) -> p o c", o=1).broadcast_to([n, 1, w])

        def pool_affine(dst3, src3, imul, iadd, n, w):
            # dst = src * sc[imul] + sc[iadd] via two Pool tensor_tensor ops
            nc.gpsimd.tensor_tensor(dst3, src3, ccol(imul, n, w), ALU.mult)
            nc.gpsimd.tensor_tensor(dst3, dst3, ccol(iadd, n, w), ALU.add)

        def poly_sin(dst, z2buf, zbuf, tmp, n, w):
            # Pool-engine sin(pi*z) poly: (((c7*z2+c5)*z2+c3)*z2+c1)*z
            t3 = tmp[:].rearrange("p (o c) -> p o c", o=1)
            z23 = z2buf[:].rearrange("p (o c) -> p o c", o=1)
            pool_affine(t3, z23, 0, 1, n, w)
            nc.gpsimd.tensor_tensor(t3, t3, z23, ALU.mult)
            nc.gpsimd.tensor_tensor(t3, t3, ccol(2, n, w), ALU.add)
            nc.gpsimd.tensor_tensor(t3, t3, z23, ALU.mult)
            nc.gpsimd.tensor_tensor(t3, t3, ccol(3, n, w), ALU.add)
            nc.gpsimd.tensor_tensor(dst[:].rearrange("p (o c) -> p o c", o=1),
                                    t3, zbuf[:].rearrange("p (o c) -> p o c", o=1),
                                    ALU.mult)

        for t in range(2):
            data = sbuf([96, _D_W], f"data{t}")
            nc.sync.dma_start(data[:], P_data[t, :, :])
            crow = data[:, _D_CROW:_D_CROW + 144]
            ctr = data[:, _D_CTR:_D_CTR + 3]
            spm = data[:, _D_SPM:_D_SPM + 192]
            oh8 = data[:, _D_OH8:_D_OH8 + 8]

            # ---- distances ----
            sqa = sbuf([96, 48], f"sqa{t}")
            sqb = sbuf([96, 48], f"sqb{t}")
            d2 = sbuf([96, 48], f"d2{t}")
            nc.scalar.activation(sqa[:], crow[:, 0:48], AF.Square,
                                 bias=ctr[:, 0:1], scale=-1.0)
            nc.scalar.activation(sqb[:], crow[:, 48:96], AF.Square,
                                 bias=ctr[:, 1:2], scale=-1.0)
            nc.vector.tensor_tensor(d2[:], sqa[:], sqb[:], ALU.add)
            nc.scalar.activation(sqa[:], crow[:, 96:144], AF.Square,
                                 bias=ctr[:, 2:3], scale=-1.0)
            nc.vector.tensor_tensor(d2[:], d2[:], sqa[:], ALU.add)
            m2 = sqb
            nc.vector.tensor_scalar(m2[:], d2[:], 1e-12, None, ALU.max, ALU.bypass)
            ln2 = sbuf([96, 48], f"ln2{t}")
            dr = sbuf([96, 48], f"dr{t}")
            nc.scalar.activation(ln2[:], m2[:], AF.Ln, bias=0.0, scale=1.0)
            nc.scalar.activation(dr[:], ln2[:], AF.Exp, bias=0.0, scale=0.5)
            selfm = sbuf([96, 48], f"selfm{t}")
            nc.vector.tensor_scalar(selfm[:], d2[:], EPS2, None, ALU.is_ge, ALU.bypass)

            # ---- radial ----
            zz = sbuf([96, 48], f"zz{t}")
            z2 = sbuf([96, 48], f"z2{t}")
            h = sbuf([96, 48], f"h{t}")
            ptmp = sbuf([96, 48], f"ptmp{t}")
            zz3 = zz[:].rearrange("p (o c) -> p o c", o=1)
            nc.vector.tensor_scalar(zz[:], dr[:], RCR, None, ALU.min, ALU.bypass)
            pool_affine(zz3, zz3, 4, 5, 96, 48)
            nc.gpsimd.tensor_tensor(z2[:], zz[:], zz[:], ALU.mult)
            poly_sin(h, z2, zz, ptmp, 96, 48)
            wr = sbuf([96, 48], f"wr{t}")
            wr3 = wr[:].rearrange("p (o c) -> p o c", o=1)
            pool_affine(wr3, h[:].rearrange("p (o c) -> p o c", o=1), 7, 6, 96, 48)
            nc.gpsimd.tensor_tensor(wr[:], wr[:], selfm[:], ALU.mult)
            wrb = sbuf([96, 48], f"wrb{t}", bf16)
            nc.gpsimd.tensor_copy(wrb[:], wr[:])
            oh8b = sbuf([96, 8], f"oh8b{t}", bf16)
            nc.gpsimd.tensor_copy(oh8b[:], oh8)

            rp = sbuf([96, 768], f"rp{t}")
            rp3 = rp[:].rearrange("p (f j) -> p f j", f=16)
            drb = dr[:].rearrange("p (o j) -> p o j", o=1).broadcast_to([96, 16, 48])
            shb = shrow.rearrange("p (f j) -> p f j", f=16)
            nc.vector.tensor_tensor(rp3, drb, shb, ALU.subtract)
            nc.scalar.activation(rp[:], rp[:], AF.Square, bias=0.0, scale=1.0)
            rpb = sbuf([96, 768], f"rpb{t}", bf16)
            nc.scalar.activation(rpb[:], rp[:], AF.Exp, bias=0.0, scale=-ETA_R)
            rpb3 = rpb[:].rearrange("p (f j) -> p f j", f=16)
            wrb3 = wrb[:].rearrange("p (o j) -> p o j", o=1).broadcast_to([96, 16, 48])
            nc.vector.tensor_tensor(rpb3, rpb3, wrb3, ALU.mult)
            rps0 = pbank(0, 8, 384)
            rps1 = pbank(0, 8, 384)
            nc.tensor.matmul(rps0, oh8b[:], rpb[:, 0:384], start=True, stop=True)
            nc.tensor.matmul(rps1, oh8b[:], rpb[:, 384:768], start=True, stop=True)
            rsb = sbuf([8, 768], f"rsb{t}")
            nc.vector.tensor_copy(rsb[:, 0:384], rps0)
            nc.scalar.copy(rsb[:, 384:768], rps1)
            nc.sync.dma_start(P_outr[t, :, :], rsb[:])

            # ---- neighbor selection ----
            t1 = sbuf([96, 48], f"t1{t}")
            w = sbuf([96, 48], f"w{t}")
            nc.vector.tensor_scalar(t1[:], d2[:], RCA * RCA, None, ALU.is_le, ALU.bypass)
            nc.vector.tensor_scalar(w[:], d2[:], -1.0, CSEL, ALU.mult, ALU.add)
            nc.vector.tensor_tensor(w[:], w[:], t1[:], ALU.mult)
            nc.vector.tensor_tensor(w[:], w[:], selfm[:], ALU.mult)
            key = sbuf([96, 48], f"key{t}")
            keyb = sbuf([96, 48], f"keyb{t}")
            mv8 = sbuf([96, 32], f"mv8{t}")
            mi8 = sbuf([96, 32], f"mi8{t}", u32)
            for s in range(4):
                kb = key if s % 2 == 0 else keyb
                nc.vector.tensor_tensor(kb[:], spm[:, 48 * s:48 * s + 48], w[:],
                                        ALU.mult)
                nc.vector.max(mv8[:, 8 * s:8 * s + 8], kb[:])
                nc.vector.max_index(mi8[:, 8 * s:8 * s + 8],
                                    mv8[:, 8 * s:8 * s + 8], kb[:])
            jlf = sbuf([96, 32], f"jlf{t}")
            nc.vector.tensor_copy(jlf[:], mi8[:])
            jlc = sbuf([96, 20], f"jlc{t}")
            mvc = sbuf([96, 20], f"mvc{t}")
            jv4 = jlf[:].rearrange("p (s q) -> p s q", s=4)
            mv4 = mv8[:].rearrange("p (s q) -> p s q", s=4)
            nc.vector.tensor_copy(jlc[:].rearrange("p (s q) -> p s q", s=4),
                                  jv4[:, :, 0:5])
            nc.vector.tensor_copy(mvc[:].rearrange("p (s q) -> p s q", s=4),
                                  mv4[:, :, 0:5])

            # ---- slot-space channels ([96, 20]) ----
            stile1 = sbuf([96, 84], f"stile1{t}")
            stile2 = sbuf([96, 84], f"stile2{t}")
            nc.gpsimd.memset(stile1[:], 0.0)
            nc.gpsimd.memset(stile2[:], 0.0)
            ds = stile1[:, 0:20]
            sq2c = stile1[:, 32:52]
            d2s = sbuf([96, 20], f"d2s{t}")
            valid = sbuf([96, 20], f"valid{t}")
            nc.vector.tensor_scalar(d2s[:], mvc[:], -1.0, CSEL, ALU.mult, ALU.add)
            nc.vector.tensor_scalar(valid[:], mvc[:], 0.5, None, ALU.is_ge, ALU.bypass)
            lnd = sbuf([96, 20], f"lnd{t}")
            nc.scalar.activation(lnd[:], d2s[:], AF.Ln, bias=0.0, scale=1.0)
            nc.scalar.activation(ds, lnd[:], AF.Exp, bias=0.0, scale=0.5)
            ivd = sbuf([96, 20], f"ivd{t}")
            nc.scalar.activation(ivd[:], lnd[:], AF.Exp, bias=0.0, scale=-0.5)
            nc.vector.tensor_scalar(sq2c, d2s[:], 2.0, None, ALU.mult, ALU.bypass)

            # gather x,y,z of selected neighbors
            i3 = sbuf([96, 20 * 48], f"i3{t}")
            i33 = i3[:].rearrange("p (k j) -> p k j", j=48)
            jlb = jlc[:].rearrange("p (k o) -> p k o", o=1).broadcast_to([96, 20, 48])
            jrb = jrow.rearrange("p (o j) -> p o j", o=1).broadcast_to([96, 20, 48])
            nc.vector.tensor_tensor(i33, jlb, jrb, ALU.is_equal)
            utile = sbuf([96, 84], f"utile{t}")
            nc.gpsimd.memset(utile[:], 0.0)
            mbufa = sbuf([96, 20 * 48], f"mbufa{t}")
            mbufb = sbuf([96, 20 * 48], f"mbufb{t}")
            g3 = sbuf([96, 20], f"g3{t}")
            for c in range(3):
                eng = nc.vector if c == 0 else nc.gpsimd
                mb = (mbufa if c % 2 == 0 else mbufb)
                mb3 = mb[:].rearrange("p (k j) -> p k j", j=48)
                cb = crow[:, 48 * c:48 * c + 48].rearrange(
                    "p (o j) -> p o j", o=1).broadcast_to([96, 20, 48])
                eng.tensor_tensor(mb3, i33, cb, ALU.mult)
                nc.vector.tensor_reduce(g3[:].rearrange("p (k o) -> p k o", o=1),
                                        mb3, mybir.AxisListType.X, ALU.add)
                uc = utile[:, 32 * c:32 * c + 20]
                nc.vector.tensor_scalar(uc, g3[:], ctr[:, c:c + 1], None,
                                        ALU.subtract, ALU.bypass)
                nc.vector.tensor_tensor(uc, uc, ivd[:], ALU.mult)

            # fc_a poly + LW channels
            az = sbuf([96, 20], f"az{t}")
            az2 = sbuf([96, 20], f"az2{t}")
            ah = sbuf([96, 20], f"ah{t}")
            aptmp = sbuf([96, 20], f"aptmp{t}")
            az3 = az[:].rearrange("p (o c) -> p o c", o=1)
            nc.vector.tensor_scalar(az[:], ds, RCA, None, ALU.min, ALU.bypass)
            pool_affine(az3, az3, 8, 5, 96, 20)
            nc.gpsimd.tensor_tensor(az2[:], az[:], az[:], ALU.mult)
            poly_sin(ah, az2, az, aptmp, 96, 20)
            kh = sbuf([96, 20], f"kh{t}")
            kh3 = kh[:].rearrange("p (o c) -> p o c", o=1)
            nc.gpsimd.tensor_tensor(kh3, ah[:].rearrange("p (o c) -> p o c", o=1),
                                    ccol(5, 96, 20), ALU.mult)
            nc.gpsimd.tensor_tensor(kh3, kh3, ccol(9, 96, 20), ALU.add)
            nc.vector.tensor_scalar(kh[:], kh[:], 1e-30, None, ALU.max, ALU.bypass)
            lnfc = sbuf([96, 20], f"lnfc{t}")
            nc.scalar.activation(lnfc[:], kh[:], AF.Ln, bias=0.0, scale=1.0)
            vkill = sbuf([96, 20], f"vkill{t}")
            nc.vector.tensor_scalar(vkill[:], valid[:], 1e4, -1e4, ALU.mult, ALU.add)
            nc.vector.tensor_tensor(lnfc[:], lnfc[:], vkill[:], ALU.add)
            lwt = sbuf([96, 20], f"lwt{t}")
            lwtb = sbuf([96, 20], f"lwtb{t}")
            lwdst = [stile1[:, 64:84], stile2[:, 0:20],
                     stile2[:, 32:52], stile2[:, 64:84]]
            for m in range(4):
                tb = lwt if m % 2 == 0 else lwtb
                if m % 2 == 0:
                    nc.vector.tensor_scalar(tb[:], ds, -2.0 * float(SHFA[m]),
                                            None, ALU.add, ALU.bypass)
                    nc.vector.tensor_tensor(tb[:], tb[:], tb[:], ALU.mult)
                    nc.vector.scalar_tensor_tensor(lwdst[m], tb[:], -2.0,
                                                   lnfc[:], ALU.mult, ALU.add)
                else:
                    tb3 = tb[:].rearrange("p (o c) -> p o c", o=1)
                    nc.gpsimd.tensor_tensor(
                        tb3, ds.rearrange("p (o c) -> p o c", o=1),
                        ccol(13 + m, 96, 20), ALU.add)
                    nc.gpsimd.tensor_tensor(tb[:], tb[:], tb[:], ALU.mult)
                    nc.gpsimd.tensor_tensor(tb3, tb3, ccol(17, 96, 20), ALU.mult)
                    nc.gpsimd.tensor_tensor(lwdst[m].rearrange(
                        "p (o c) -> p o c", o=1), tb3,
                        lnfc[:].rearrange("p (o c) -> p o c", o=1), ALU.add)

            # transpose to SD
            for src_, dst in ((utile, SDu), (stile1, SDs1), (stile2, SDs2)):
                tp = pbank(0, 84, 96)
                nc.tensor.transpose(tp, src_[:], ident)
                nc.vector.tensor_copy(dst[:, 96 * t:96 * t + 96], tp)

        # ---- pair space ----
        shh2 = sbuf([CW, 384], "shh2")
        b4 = sbuf([CW, 384], "b4")
        lwb = [sbuf([CW, 384], f"lwb{m}") for m in range(4)]
        for ci in range(2):
            c0 = CW * ci
            cs = slice(192 * ci, 192 * ci + 192)
            vd = [pbank(0, CW, 192) for c in range(3)]
            for c in range(3):
                nc.tensor.matmul(vd[c], expd_at(32 * c, c0),
                                 SDu[32 * c:32 * c + 20, :], start=True, stop=True)
            pS = pbank(0, CW, 192)
            pQ = pbank(0, CW, 192)
            nc.tensor.matmul(pS, exps_at(0, c0), SDs1[0:20, :],
                             start=True, stop=True)
            nc.tensor.matmul(pQ, exps_at(32, c0), SDs1[32:52, :],
                             start=True, stop=True)
            pL = [pbank(0, CW, 192) for m in range(4)]
            srcL = [SDs1[64:84, :], SDs2[0:20, :], SDs2[32:52, :], SDs2[64:84, :]]
            basL = [64, 0, 32, 64]
            for m in range(4):
                nc.tensor.matmul(pL[m], exps_at(basL[m], c0), srcL[m],
                                 start=True, stop=True)
            # shh2 = sum_c vd_c^2 (chunk column block)
            tq = sbuf([CW, 192], f"tq{ci}")
            tq2 = sbuf([CW, 192], f"tq2{ci}")
            nc.scalar.activation(shh2[:, cs], vd[0], AF.Square, bias=0.0, scale=1.0)
            nc.scalar.activation(tq[:], vd[1], AF.Square, bias=0.0, scale=1.0)
            nc.scalar.activation(tq2[:], vd[2], AF.Square, bias=0.0, scale=1.0)
            nc.vector.tensor_tensor(shh2[:, cs], shh2[:, cs], tq[:], ALU.add)
            nc.vector.tensor_tensor(shh2[:, cs], shh2[:, cs], tq2[:], ALU.add)
            # b4 = SQ2' - 2*S^2
            s2 = sbuf([CW, 192], f"s2{ci}")
            nc.scalar.activation(s2[:], pS, AF.Square, bias=0.0, scale=1.0)
            nc.vector.scalar_tensor_tensor(b4[:, cs], s2[:], -2.0, pQ,
                                           ALU.mult, ALU.add)
            for m in range(4):
                nc.vector.tensor_tensor(lwb[m][:, cs], pL[m], b4[:, cs], ALU.add)

        nc.vector.tensor_scalar(shh2[:], shh2[:], 0.0, 4.0, ALU.max, ALU.min)
        b975 = sbuf([128, 1], "b975")
        b025 = sbuf([128, 1], "b025")
        nc.vector.memset(b975[:], 0.975)
        nc.vector.memset(b025[:], 0.025)
        lnc = sbuf([CW, 384], "lnc")
        lns = sbuf([CW, 384], "lns")
        chp = sbuf([CW, 384], "chp")
        shp = sbuf([CW, 384], "shp")
        nc.scalar.activation(lnc[:], shh2[:], AF.Ln, bias=b975[0:CW, :],
                             scale=-0.2375)
        nc.scalar.activation(chp[:], lnc[:], AF.Exp, bias=0.0, scale=0.5)
        nc.scalar.activation(lns[:], shh2[:], AF.Ln, bias=b025[0:CW, :],
                             scale=0.2375)
        nc.scalar.activation(shp[:], lns[:], AF.Exp, bias=0.0, scale=0.5)
        wm = []
        for m in range(4):
            w_ = sbuf([CW, 384], f"wm{m}", bf16)
            nc.scalar.activation(w_[:], lwb[m][:], AF.Exp,
                                 bias=CF[0:CW, _C_BW + m:_C_BW + m + 1], scale=1.0)
            wm.append(w_)
        # terms: one big tile [95, 32*384] bf16, mz = m*8+z at cols mz*384
        terms = sbuf([CW, 32 * 384], "terms", bf16)
        tv = sbuf([CW, 384], "tv")
        tvb = sbuf([CW, 384], "tvb")
        f1 = sbuf([CW, 384], "f1a", bf16)
        f1b = sbuf([CW, 384], "f1b", bf16)
        for z in range(8):
            tanz = float(np.tan(SHFZ[z] / 2))
            tb = tv if z % 2 == 0 else tvb
            fb = f1 if z % 2 == 0 else f1b
            nc.vector.scalar_tensor_tensor(tb[:], shp[:], tanz, chp[:],
                                           ALU.mult, ALU.add)
            nc.scalar.activation(tb[:], tb[:], AF.Ln, bias=0.0, scale=1.0)
            nc.scalar.activation(fb[:], tb[:], AF.Exp,
                                 bias=CF[0:CW, _C_BZ + z:_C_BZ + z + 1], scale=64.0)
            for m in range(4):
                mz = m * 8 + z
                eng2 = nc.gpsimd if (mz % 4 == 3) else nc.vector
                eng2.tensor_tensor(terms[:, 384 * mz:384 * mz + 384],
                                   wm[m][:], fb[:], ALU.mult)

        # bucket contraction: per g, rhs = 2 mz terms strided view
        t4 = terms[:].rearrange("p (mz ci at) -> p mz ci at", mz=32, ci=2)
        for g in range(16):
            pA = pbank(0, 10, 384)
            for ci in range(2):
                nc.tensor.matmul(pA, buckb[:, 10 * ci:10 * ci + 10],
                                 t4[:, 2 * g:2 * g + 2, ci, :],
                                 start=(ci == 0), stop=(ci == 1))
            asb = sbuf([10, 384], f"asb{g % 4}")
            if g % 2:
                nc.scalar.copy(asb[:], pA)
            else:
                nc.vector.tensor_copy(asb[:], pA)
            nc.sync.dma_start(P_outa[g, :, :], asb[:])

    nc.compile()
    return nc


def _pack_inputs(species, coords):
    sp = species.astype(np.int64)
    co = coords.astype(np.float32)
    in_maps = []
    cp = _BUILT["cp"]
    for c in range(8):
        data = np.zeros((2, 96, _D_W), np.float32)
        for t in range(2):
            for m in range(2):
                n = 4 * c + 2 * t + m
                rows = slice(48 * m, 48 * m + 48)
                data[t, rows, _D_CROW:_D_CROW + 144] = co[n].T.reshape(-1)[None, :]
                data[t, rows, _D_CTR:_D_CTR + 3] = co[n]
                for s in range(4):
                    data[t, rows, _D_SPM + 48 * s:_D_SPM + 48 * s + 48] = \
                        (sp[n] == s).astype(np.float32)[None, :]
                for s in range(4):
                    col = _D_OH8 + 4 * m + s
                    data[t, rows, col] = (sp[n] == s).astype(np.float32)
        in_maps.append(dict(data=data, consts=cp))
    return in_maps


def kernel(species, coordinates):
    import sys
    sys.path.insert(0, "/opt/trn_rl_repo")
    from concourse.bass_utils import run_bass_kernel_spmd

    species = np.asarray(species)
    coords = np.asarray(coordinates, dtype=np.float32)
    N = species.shape[0]
    if "nc" not in _BUILT:
        _BUILT["cp"] = _constpack()
        _BUILT["nc"] = _build()
    nc = _BUILT["nc"]

    in_maps = _pack_inputs(species, coords)
    res = run_bass_kernel_spmd(nc, in_maps, list(range(8)))
    full = np.zeros((N, 48, 384), np.float32)
    for c in range(8):
        outr = np.asarray(res.results[c]["outr"]).reshape(2, 2, 4, 16, 48)
        outa = np.asarray(res.results[c]["outa"]).reshape(16, 10, 2, 4, 48)
        # radial: outr[t, m, s, f, j] -> mol 2t+m, atom j, col s*16+f
        rad = outr.transpose(0, 1, 4, 2, 3).reshape(4, 48, 64)
        full[4 * c:4 * c + 4, :, 0:64] = rad
        # angular: outa[g, p, k, mol', j]: at = 192-atom index = (t*2+m')*48+j
        # mz = 2g+k; feature col = 64 + p*32 + mz
        ang = outa.transpose(3, 4, 1, 0, 2).reshape(4, 48, 10, 32)
        full[4 * c:4 * c + 4, :, 64:384] = ang.reshape(4, 48, 320)
    return full
